# revision 3
# baseline (speedup 1.0000x reference)
"""Trainium2 Bass kernel for nn_KANStressPredictor (fp16 planes version).

Per element-triple (s0, s1, s2) of `strain` [B, T, 3]:
    t1, t2 = eigenvalues of C = (s0+s1+1) -/+ rad, rad = sqrt((s0-s1)^2+s2^2)
    out0, out1 = exp(ki0/3 * (l_i - 0.5*l_other)),  l_i = ln(t_i)
    out2       = ki1 * 0.5 * (l1 + l2)

Layout: host deinterleaves strain into three fp16 planes a, b, c of shape
[128, 8192] per core (data-parallel over batch across 8 cores) and upcasts
the fp16 device outputs to f32. Unit-stride 16-bit planes get the DVE 2x
performance mode and halve HBM traffic vs f32.

Structure (per 2048-triple chunk):
  Phase A (sqrt table):  u = a-b, u2 = u*u (DVE), q = c*c (GPSIMD),
      r2 = u2+q (DVE), +rad = Sqrt(r2) (ACT), -rad = -1*+rad (DVE TS 4x),
      st = a+b (DVE).  rs tile holds [-rad || +rad].
  Phase B (ln/exp table):  dd = st_broadcast + rs  -> [d1 || d2] in ONE
      2x-mode TT; l = ln(dd+1) in-place (one ACT op); w block via one stt
      against the half-swapped view; o0/o1 = exp(w) written interleaved
      (one ACT op); L = l1+l2 (GPSIMD); o2 = 0.5*ki1*L (DVE TS 4x).

Emission is software-pipelined (ln of chunk j+1 is enqueued before exp of
chunk j) so the in-order ACT queue never stalls on DVE. The sqrt table is
used only in phase A and ln/exp only in phase B: one table switch per pass.
"""

import contextlib
import sys

for _p in ("/opt/trn_rl_repo",):
    if _p not in sys.path:
        sys.path.insert(0, _p)

import numpy as np

import concourse.bacc as bacc
import concourse.bass as bass
import concourse.tile as tile
from concourse import mybir
from concourse.bass_utils import run_bass_kernel_spmd

N_CORES = 8
P = 128
F = 8192            # triples per partition per core
CT = 2048           # chunk size (triples) for both phases

f32 = mybir.dt.float32
f16 = mybir.dt.float16
bf16 = mybir.dt.bfloat16

_cache: dict = {}


def _build(ki0: float, ki1: float, loop_reps: int = 1, use_gpsimd: bool = True):
    key = (ki0, ki1, loop_reps, use_gpsimd)
    if key in _cache:
        return _cache[key]

    AF = mybir.ActivationFunctionType
    Add = mybir.AluOpType.add
    Sub = mybir.AluOpType.subtract
    Mult = mybir.AluOpType.mult

    n = F // CT

    nc = bacc.Bacc("TRN2", target_bir_lowering=False, debug=False)
    a_ap = nc.dram_tensor("a", [P, F], f16, kind="ExternalInput").ap()
    b_ap = nc.dram_tensor("b", [P, F], f16, kind="ExternalInput").ap()
    c_ap = nc.dram_tensor("c", [P, F], f16, kind="ExternalInput").ap()
    op_ap = nc.dram_tensor("op", [P, 2 * F], f16, kind="ExternalOutput").ap()
    o2_ap = nc.dram_tensor("o2", [P, F], f16, kind="ExternalOutput").ap()

    with tile.TileContext(nc) as tc:
        with (
            tc.tile_pool(name="persist", bufs=1) as pp,
            tc.tile_pool(name="pa", bufs=2) as pa,
            tc.tile_pool(name="pb", bufs=2) as pb,
            tc.tile_pool(name="io", bufs=2) as iop,
        ):
            stk = [pp.tile([P, CT], f16, name=f"st{i}", tag=f"st{i}")
                   for i in range(n)]
            rsk = [pp.tile([P, 2 * CT], bf16, name=f"rs{i}", tag=f"rs{i}")
                   for i in range(n)]

            loop = tc.For_i(0, loop_reps) if loop_reps > 1 else \
                contextlib.nullcontext()
            with loop:
                # ---- Phase A: inputs -> [-rad || +rad], st (sqrt table) ----
                def a_chunk(ci):
                    sl = bass.ts(ci, CT)
                    A = iop.tile([P, CT], f16, name="a", tag="a")
                    B = iop.tile([P, CT], f16, name="b", tag="b")
                    C = iop.tile([P, CT], f16, name="c", tag="c")
                    nc.sync.dma_start(A[:], a_ap[:, sl])
                    nc.sync.dma_start(B[:], b_ap[:, sl])
                    nc.sync.dma_start(C[:], c_ap[:, sl])
                    q = pa.tile([P, CT], bf16, name="q", tag="q")
                    if use_gpsimd:
                        nc.gpsimd.tensor_tensor(q[:], C[:], C[:], Mult)
                    else:
                        nc.vector.tensor_tensor(q[:], C[:], C[:], Mult)
                    u = pa.tile([P, CT], f16, name="u", tag="u")
                    nc.vector.tensor_tensor(u[:], A[:], B[:], Sub)
                    u2 = pa.tile([P, CT], bf16, name="u2", tag="u2")
                    nc.vector.tensor_tensor(u2[:], u[:], u[:], Mult)
                    r2 = pa.tile([P, CT], bf16, name="r2", tag="r2")
                    nc.vector.tensor_tensor(r2[:], u2[:], q[:], Add)
                    nc.vector.tensor_tensor(stk[ci][:], A[:], B[:], Add)
                    nc.scalar.activation(rsk[ci][:][:, CT:2 * CT], r2[:],
                                         AF.Sqrt)

                def a_neg(ci):
                    RS = rsk[ci][:]
                    nc.vector.tensor_scalar_mul(RS[:, 0:CT], RS[:, CT:2 * CT],
                                                -1.0)

                for ci in range(n):
                    a_chunk(ci)
                    if ci >= 1:
                        a_neg(ci - 1)
                a_neg(n - 1)

                # ---- Phase B: st, rs -> outputs (ln/exp table) ----
                dds = {}

                def b_front(j):
                    dd = pb.tile([P, 2 * CT], f16, name="dd", tag="dd")
                    dds[j] = dd
                    spair = stk[j][:].unsqueeze(1).broadcast_to([P, 2, CT])
                    rsv = rsk[j][:].rearrange("p (k n) -> p k n", k=2)
                    ddv = dd[:].rearrange("p (k n) -> p k n", k=2)
                    nc.vector.tensor_tensor(ddv, spair, rsv, Add)
                    # l block, in place: [l1 || l2]
                    nc.scalar.activation(dd[:], dd[:], AF.Ln, bias=1.0)

                def b_back(j):
                    dd = dds.pop(j)
                    ddv = dd[:].rearrange("p (k n) -> p k n", k=2)
                    lsw = ddv[:, ::-1]
                    W = pb.tile([P, 2 * CT], f16, name="w", tag="w")
                    Wv = W[:].rearrange("p (k n) -> p k n", k=2)
                    nc.vector.scalar_tensor_tensor(Wv, lsw, -0.5, ddv, Mult,
                                                   Add)
                    OP = iop.tile([P, 2 * CT], f16, name="opt", tag="opt")
                    OPv = OP[:].rearrange("p (n k) -> p k n", k=2)
                    nc.scalar.activation(OPv, Wv, AF.Exp, scale=ki0 / 3.0)
                    L = pb.tile([P, CT], f16, name="L", tag="L")
                    if use_gpsimd:
                        nc.gpsimd.tensor_tensor(L[:], dd[:][:, 0:CT],
                                                dd[:][:, CT:2 * CT], Add)
                    else:
                        nc.vector.tensor_tensor(L[:], dd[:][:, 0:CT],
                                                dd[:][:, CT:2 * CT], Add)
                    O2 = iop.tile([P, CT], f16, name="o2t", tag="o2t")
                    nc.vector.tensor_scalar_mul(O2[:], L[:], ki1 * 0.5)
                    nc.sync.dma_start(op_ap[:, bass.ts(j, 2 * CT)], OP[:])
                    nc.sync.dma_start(o2_ap[:, bass.ts(j, CT)], O2[:])

                for j in range(n):
                    b_front(j)
                    if j >= 1:
                        b_back(j - 1)
                b_back(n - 1)

    nc.compile()
    _cache[key] = nc
    return nc


def _prep_inputs(strain: np.ndarray):
    """strain [B, T, 3] f32 -> per-core fp16 planes."""
    B, T, C = strain.shape
    assert C == 3 and B % N_CORES == 0
    h = strain.astype(np.float16)
    hp = h.reshape(N_CORES, P, F, 3)
    a = np.ascontiguousarray(hp[..., 0])
    b = np.ascontiguousarray(hp[..., 1])
    c = np.ascontiguousarray(hp[..., 2])
    return a, b, c


def _assemble_out(op: np.ndarray, o2: np.ndarray, B: int, T: int):
    """op [8,P,2F] f16 pairs, o2 [8,P,F] f16 -> [B,T,3] f32."""
    out = np.empty((B, T, 3), dtype=np.float32)
    ov = out.reshape(N_CORES, P, F, 3)
    ov[..., 0:2] = op.reshape(N_CORES, P, F, 2)
    ov[..., 2] = o2
    return out


def _run(strain: np.ndarray, ki0: float, ki1: float, trace: bool = False,
         use_gpsimd: bool = True):
    B, T, C = strain.shape
    a, b, c = _prep_inputs(strain)
    nc = _build(float(ki0), float(ki1), 1, use_gpsimd)
    in_maps = [{"a": a[i], "b": b[i], "c": c[i]} for i in range(N_CORES)]
    res = run_bass_kernel_spmd(nc, in_maps, list(range(N_CORES)), trace=trace)
    op = np.stack([np.asarray(res.results[i]["op"]) for i in range(N_CORES)])
    o2 = np.stack([np.asarray(res.results[i]["o2"]) for i in range(N_CORES)])
    return _assemble_out(op, o2, B, T), res


def kernel(strain: np.ndarray, ki0, ki1) -> np.ndarray:
    out, _ = _run(np.asarray(strain), float(np.asarray(ki0)),
                  float(np.asarray(ki1)))
    return out


# revision 6
# speedup vs baseline: 1.0246x; 1.0246x over previous
"""Trainium2 Bass kernel for nn_KANStressPredictor (fp16 planes version).

Per element-triple (s0, s1, s2) of `strain` [B, T, 3]:
    t1, t2 = eigenvalues of C = (s0+s1+1) -/+ rad, rad = sqrt((s0-s1)^2+s2^2)
    out0, out1 = exp(ki0/3 * (l_i - 0.5*l_other)),  l_i = ln(t_i)
    out2       = ki1 * 0.5 * (l1 + l2)

Layout: host deinterleaves strain into three fp16 planes a, b, c of shape
[128, 8192] per core (data-parallel over batch across 8 cores) and upcasts
the fp16 device outputs to f32. Unit-stride 16-bit planes get the DVE 2x
performance mode and halve HBM traffic vs f32.

Structure (per 2048-triple chunk):
  Phase A (sqrt table):  u = a-b, u2 = u*u (DVE), q = c*c (GPSIMD),
      r2 = u2+q (DVE), +rad = Sqrt(r2) (ACT), -rad = -1*+rad (DVE TS 4x),
      st = a+b (DVE).  rs tile holds [-rad || +rad].
  Phase B (ln/exp table):  dd = st_broadcast + rs  -> [d1 || d2] in ONE
      2x-mode TT; l = ln(dd+1) in-place (one ACT op); w block via one stt
      against the half-swapped view; o0/o1 = exp(w) written interleaved
      (one ACT op); L = l1+l2 (GPSIMD); o2 = 0.5*ki1*L (DVE TS 4x).

Emission is software-pipelined (ln of chunk j+1 is enqueued before exp of
chunk j) so the in-order ACT queue never stalls on DVE. The sqrt table is
used only in phase A and ln/exp only in phase B: one table switch per pass.
"""

import contextlib
import sys

for _p in ("/opt/trn_rl_repo",):
    if _p not in sys.path:
        sys.path.insert(0, _p)

import numpy as np

import concourse.bacc as bacc
import concourse.bass as bass
import concourse.tile as tile
from concourse import mybir
from concourse.bass_utils import run_bass_kernel_spmd

N_CORES = 8
P = 128
F = 8192            # triples per partition per core
CT = 2048           # chunk size (triples) for both phases

f32 = mybir.dt.float32
f16 = mybir.dt.float16
bf16 = mybir.dt.bfloat16

_cache: dict = {}


def _build(ki0: float, ki1: float, loop_reps: int = 1, use_gpsimd: bool = True,
           bcast: bool = False, inplace_ln: bool = True, pool_L: bool = False):
    key = (ki0, ki1, loop_reps, use_gpsimd, bcast, inplace_ln, pool_L)
    if key in _cache:
        return _cache[key]

    AF = mybir.ActivationFunctionType
    Add = mybir.AluOpType.add
    Sub = mybir.AluOpType.subtract
    Mult = mybir.AluOpType.mult

    n = F // CT

    nc = bacc.Bacc("TRN2", target_bir_lowering=False, debug=False)
    a_ap = nc.dram_tensor("a", [P, F], f16, kind="ExternalInput").ap()
    b_ap = nc.dram_tensor("b", [P, F], f16, kind="ExternalInput").ap()
    c_ap = nc.dram_tensor("c", [P, F], f16, kind="ExternalInput").ap()
    op_ap = nc.dram_tensor("op", [P, 2 * F], f16, kind="ExternalOutput").ap()
    o2_ap = nc.dram_tensor("o2", [P, F], f16, kind="ExternalOutput").ap()

    with tile.TileContext(nc) as tc:
        with (
            tc.tile_pool(name="persist", bufs=1) as pp,
            tc.tile_pool(name="pa", bufs=2) as pa,
            tc.tile_pool(name="pb", bufs=2) as pb,
            tc.tile_pool(name="io", bufs=2) as iop,
        ):
            stk = [pp.tile([P, CT], f16, name=f"st{i}", tag=f"st{i}")
                   for i in range(n)]
            rsk = [pp.tile([P, 2 * CT], bf16, name=f"rs{i}", tag=f"rs{i}")
                   for i in range(n)]

            loop = tc.For_i(0, loop_reps) if loop_reps > 1 else \
                contextlib.nullcontext()
            with loop:
                # ---- Phase A: inputs -> [-rad || +rad], st (sqrt table) ----
                def a_chunk(ci):
                    sl = bass.ts(ci, CT)
                    A = iop.tile([P, CT], f16, name="a", tag="a")
                    B = iop.tile([P, CT], f16, name="b", tag="b")
                    C = iop.tile([P, CT], f16, name="c", tag="c")
                    nc.sync.dma_start(A[:], a_ap[:, sl])
                    nc.sync.dma_start(B[:], b_ap[:, sl])
                    nc.sync.dma_start(C[:], c_ap[:, sl])
                    q = pa.tile([P, CT], bf16, name="q", tag="q")
                    if use_gpsimd:
                        nc.gpsimd.tensor_tensor(q[:], C[:], C[:], Mult)
                    else:
                        nc.vector.tensor_tensor(q[:], C[:], C[:], Mult)
                    u = pa.tile([P, CT], f16, name="u", tag="u")
                    nc.vector.tensor_tensor(u[:], A[:], B[:], Sub)
                    u2 = pa.tile([P, CT], bf16, name="u2", tag="u2")
                    nc.vector.tensor_tensor(u2[:], u[:], u[:], Mult)
                    r2 = pa.tile([P, CT], bf16, name="r2", tag="r2")
                    nc.vector.tensor_tensor(r2[:], u2[:], q[:], Add)
                    nc.vector.tensor_tensor(stk[ci][:], A[:], B[:], Add)
                    nc.scalar.activation(rsk[ci][:][:, CT:2 * CT], r2[:],
                                         AF.Sqrt)

                def a_neg(ci):
                    RS = rsk[ci][:]
                    nc.vector.tensor_scalar_mul(RS[:, 0:CT], RS[:, CT:2 * CT],
                                                -1.0)

                for ci in range(n):
                    a_chunk(ci)
                    if ci >= 1:
                        a_neg(ci - 1)
                a_neg(n - 1)

                # ---- Phase B: st, rs -> outputs (ln/exp table) ----
                dds = {}

                def b_front(j):
                    dd = pb.tile([P, 2 * CT], f16, name="dd", tag="dd")
                    RS = rsk[j][:]
                    ST = stk[j][:]
                    if bcast:
                        spair = ST.unsqueeze(1).broadcast_to([P, 2, CT])
                        rsv = RS.rearrange("p (k n) -> p k n", k=2)
                        ddv = dd[:].rearrange("p (k n) -> p k n", k=2)
                        nc.vector.tensor_tensor(ddv, spair, rsv, Add)
                    else:
                        nc.vector.tensor_tensor(dd[:][:, 0:CT], ST,
                                                RS[:, 0:CT], Add)
                        nc.vector.tensor_tensor(dd[:][:, CT:2 * CT], ST,
                                                RS[:, CT:2 * CT], Add)
                    # l block: [l1 || l2]
                    if inplace_ln:
                        nc.scalar.activation(dd[:], dd[:], AF.Ln, bias=1.0)
                        dds[j] = dd
                    else:
                        lt = pb.tile([P, 2 * CT], f16, name="lt", tag="lt")
                        nc.scalar.activation(lt[:], dd[:], AF.Ln, bias=1.0)
                        dds[j] = lt

                def b_back(j):
                    dd = dds.pop(j)
                    ddv = dd[:].rearrange("p (k n) -> p k n", k=2)
                    lsw = ddv[:, ::-1]
                    W = pb.tile([P, 2 * CT], f16, name="w", tag="w")
                    Wv = W[:].rearrange("p (k n) -> p k n", k=2)
                    nc.vector.scalar_tensor_tensor(Wv, lsw, -0.5, ddv, Mult,
                                                   Add)
                    OP = iop.tile([P, 2 * CT], f16, name="opt", tag="opt")
                    OPv = OP[:].rearrange("p (n k) -> p k n", k=2)
                    nc.scalar.activation(OPv, Wv, AF.Exp, scale=ki0 / 3.0)
                    L = pb.tile([P, CT], f16, name="L", tag="L")
                    eng = nc.gpsimd if pool_L else nc.vector
                    eng.tensor_tensor(L[:], dd[:][:, 0:CT],
                                      dd[:][:, CT:2 * CT], Add)
                    O2 = iop.tile([P, CT], f16, name="o2t", tag="o2t")
                    nc.vector.tensor_scalar_mul(O2[:], L[:], ki1 * 0.5)
                    nc.sync.dma_start(op_ap[:, bass.ts(j, 2 * CT)], OP[:])
                    nc.sync.dma_start(o2_ap[:, bass.ts(j, CT)], O2[:])

                for j in range(n):
                    b_front(j)
                    if j >= 1:
                        b_back(j - 1)
                b_back(n - 1)

    nc.compile()
    _cache[key] = nc
    return nc


def _prep_inputs(strain: np.ndarray):
    """strain [B, T, 3] f32 -> per-core fp16 planes."""
    B, T, C = strain.shape
    assert C == 3 and B % N_CORES == 0
    h = strain.astype(np.float16)
    hp = h.reshape(N_CORES, P, F, 3)
    a = np.ascontiguousarray(hp[..., 0])
    b = np.ascontiguousarray(hp[..., 1])
    c = np.ascontiguousarray(hp[..., 2])
    return a, b, c


def _assemble_out(op: np.ndarray, o2: np.ndarray, B: int, T: int):
    """op [8,P,2F] f16 pairs, o2 [8,P,F] f16 -> [B,T,3] f32."""
    out = np.empty((B, T, 3), dtype=np.float32)
    ov = out.reshape(N_CORES, P, F, 3)
    ov[..., 0:2] = op.reshape(N_CORES, P, F, 2)
    ov[..., 2] = o2
    return out


def _run(strain: np.ndarray, ki0: float, ki1: float, trace: bool = False,
         use_gpsimd: bool = True):
    B, T, C = strain.shape
    a, b, c = _prep_inputs(strain)
    nc = _build(float(ki0), float(ki1), 1, use_gpsimd)
    in_maps = [{"a": a[i], "b": b[i], "c": c[i]} for i in range(N_CORES)]
    res = run_bass_kernel_spmd(nc, in_maps, list(range(N_CORES)), trace=trace)
    op = np.stack([np.asarray(res.results[i]["op"]) for i in range(N_CORES)])
    o2 = np.stack([np.asarray(res.results[i]["o2"]) for i in range(N_CORES)])
    return _assemble_out(op, o2, B, T), res


def kernel(strain: np.ndarray, ki0, ki1) -> np.ndarray:
    out, _ = _run(np.asarray(strain), float(np.asarray(ki0)),
                  float(np.asarray(ki1)))
    return out


# revision 10
# speedup vs baseline: 1.4203x; 1.3862x over previous
"""Trainium2 Bass kernel for nn_KANStressPredictor (fp16 planes version).

Per element-triple (s0, s1, s2) of `strain` [B, T, 3]:
    t1, t2 = eigenvalues of C = (s0+s1+1) -/+ rad, rad = sqrt((s0-s1)^2+s2^2)
    out0, out1 = exp(ki0/3 * (l_i - 0.5*l_other)),  l_i = ln(t_i)
    out2       = ki1 * 0.5 * (l1 + l2)

Layout: host deinterleaves strain into three fp16 planes a, b, c of shape
[128, 8192] per core (data-parallel over batch across 8 cores) and upcasts
the fp16 device outputs to f32. Unit-stride 16-bit planes get the DVE 2x
performance mode and halve HBM traffic vs f32.

Structure (per 2048-triple chunk):
  Phase A (sqrt table):  u = a-b, u2 = u*u (DVE), q = c*c (GPSIMD),
      r2 = u2+q (DVE), +rad = Sqrt(r2) (ACT), -rad = -1*+rad (DVE TS 4x),
      st = a+b (DVE).  rs tile holds [-rad || +rad].
  Phase B (ln/exp table):  dd = st_broadcast + rs  -> [d1 || d2] in ONE
      2x-mode TT; l = ln(dd+1) in-place (one ACT op); w block via one stt
      against the half-swapped view; o0/o1 = exp(w) written interleaved
      (one ACT op); L = l1+l2 (GPSIMD); o2 = 0.5*ki1*L (DVE TS 4x).

Emission is software-pipelined (ln of chunk j+1 is enqueued before exp of
chunk j) so the in-order ACT queue never stalls on DVE. The sqrt table is
used only in phase A and ln/exp only in phase B: one table switch per pass.
"""

import contextlib
import sys

for _p in ("/opt/trn_rl_repo",):
    if _p not in sys.path:
        sys.path.insert(0, _p)

import numpy as np

import concourse.bacc as bacc
import concourse.bass as bass
import concourse.tile as tile
from concourse import mybir
from concourse.bass_utils import run_bass_kernel_spmd

N_CORES = 8
P = 128
F = 8192            # triples per partition per core
CT = 2048           # chunk size (triples) for both phases

f32 = mybir.dt.float32
f16 = mybir.dt.float16
bf16 = mybir.dt.bfloat16

_cache: dict = {}


def _build(ki0: float, ki1: float, loop_reps: int = 1, use_gpsimd: bool = True,
           bcast: bool = False, inplace_ln: bool = True, pool_L: bool = False,
           pre_neg: bool = True, ctb: int = CT, pipelined: bool = True):
    key = (ki0, ki1, loop_reps, use_gpsimd, bcast, inplace_ln, pool_L,
           pre_neg, ctb, pipelined)
    if key in _cache:
        return _cache[key]

    AF = mybir.ActivationFunctionType
    Add = mybir.AluOpType.add
    Sub = mybir.AluOpType.subtract
    Mult = mybir.AluOpType.mult

    n = F // CT

    nc = bacc.Bacc("TRN2", target_bir_lowering=False, debug=False)
    a_ap = nc.dram_tensor("a", [P, F], f16, kind="ExternalInput").ap()
    b_ap = nc.dram_tensor("b", [P, F], f16, kind="ExternalInput").ap()
    c_ap = nc.dram_tensor("c", [P, F], f16, kind="ExternalInput").ap()
    op_ap = nc.dram_tensor("op", [P, 2 * F], f16, kind="ExternalOutput").ap()
    o2_ap = nc.dram_tensor("o2", [P, F], f16, kind="ExternalOutput").ap()

    with tile.TileContext(nc) as tc:
        with (
            tc.tile_pool(name="persist", bufs=1) as pp,
            tc.tile_pool(name="pa", bufs=2) as pa,
            tc.tile_pool(name="pb", bufs=2) as pb,
            tc.tile_pool(name="io", bufs=2) as iop,
        ):
            stk = [pp.tile([P, CT], f16, name=f"st{i}", tag=f"st{i}")
                   for i in range(n)]
            rw = 2 * CT if pre_neg else CT
            rsk = [pp.tile([P, rw], bf16, name=f"rs{i}", tag=f"rs{i}")
                   for i in range(n)]
            nB = F // ctb
            rB = CT // ctb

            loop = tc.For_i(0, loop_reps) if loop_reps > 1 else \
                contextlib.nullcontext()
            with loop:
                # ---- Phase A: inputs -> [-rad || +rad], st (sqrt table) ----
                def a_chunk(ci):
                    sl = bass.ts(ci, CT)
                    A = iop.tile([P, CT], f16, name="a", tag="a")
                    B = iop.tile([P, CT], f16, name="b", tag="b")
                    C = iop.tile([P, CT], f16, name="c", tag="c")
                    nc.sync.dma_start(A[:], a_ap[:, sl])
                    nc.sync.dma_start(B[:], b_ap[:, sl])
                    nc.sync.dma_start(C[:], c_ap[:, sl])
                    q = pa.tile([P, CT], bf16, name="q", tag="q")
                    if use_gpsimd:
                        nc.gpsimd.tensor_tensor(q[:], C[:], C[:], Mult)
                    else:
                        nc.vector.tensor_tensor(q[:], C[:], C[:], Mult)
                    u = pa.tile([P, CT], f16, name="u", tag="u")
                    nc.vector.tensor_tensor(u[:], A[:], B[:], Sub)
                    u2 = pa.tile([P, CT], bf16, name="u2", tag="u2")
                    nc.vector.tensor_tensor(u2[:], u[:], u[:], Mult)
                    r2 = pa.tile([P, CT], bf16, name="r2", tag="r2")
                    nc.vector.tensor_tensor(r2[:], u2[:], q[:], Add)
                    nc.vector.tensor_tensor(stk[ci][:], A[:], B[:], Add)
                    rad_dst = rsk[ci][:][:, CT:2 * CT] if pre_neg \
                        else rsk[ci][:]
                    nc.scalar.activation(rad_dst, r2[:], AF.Sqrt)

                def a_neg(ci):
                    RS = rsk[ci][:]
                    nc.vector.tensor_scalar_mul(RS[:, 0:CT], RS[:, CT:2 * CT],
                                                -1.0)

                for ci in range(n):
                    a_chunk(ci)
                    if pre_neg and ci >= 1:
                        a_neg(ci - 1)
                if pre_neg:
                    a_neg(n - 1)

                # ---- Phase B: st, rs -> outputs (ln/exp table) ----
                dds = {}

                def b_front(j):
                    ca, co = j // rB, (j % rB) * ctb
                    dd = pb.tile([P, 2 * ctb], f16, name="dd", tag="dd")
                    ST = stk[ca][:][:, co:co + ctb]
                    if bcast:
                        RSv = rsk[ca][:].rearrange(
                            "p (k n) -> p k n", k=2)[:, :, co:co + ctb]
                        spair = ST.unsqueeze(1).broadcast_to([P, 2, ctb])
                        ddv = dd[:].rearrange("p (k n) -> p k n", k=2)
                        nc.vector.tensor_tensor(ddv, spair, RSv, Add)
                    elif pre_neg:
                        RS = rsk[ca][:]
                        nc.vector.tensor_tensor(
                            dd[:][:, 0:ctb], ST, RS[:, co:co + ctb], Add)
                        nc.vector.tensor_tensor(
                            dd[:][:, ctb:2 * ctb], ST,
                            RS[:, CT + co:CT + co + ctb], Add)
                    else:
                        RD = rsk[ca][:][:, co:co + ctb]
                        nc.vector.tensor_tensor(dd[:][:, 0:ctb], ST, RD, Sub)
                        nc.vector.tensor_tensor(dd[:][:, ctb:2 * ctb], ST, RD,
                                                Add)
                    # l block: [l1 || l2]
                    if inplace_ln:
                        nc.scalar.activation(dd[:], dd[:], AF.Ln, bias=1.0)
                        dds[j] = dd
                    else:
                        lt = pb.tile([P, 2 * ctb], f16, name="lt", tag="lt")
                        nc.scalar.activation(lt[:], dd[:], AF.Ln, bias=1.0)
                        dds[j] = lt

                def b_back(j):
                    dd = dds.pop(j)
                    ddv = dd[:].rearrange("p (k n) -> p k n", k=2)
                    lsw = ddv[:, ::-1]
                    W = pb.tile([P, 2 * ctb], f16, name="w", tag="w")
                    Wv = W[:].rearrange("p (k n) -> p k n", k=2)
                    nc.vector.scalar_tensor_tensor(Wv, lsw, -0.5, ddv, Mult,
                                                   Add)
                    OP = iop.tile([P, 2 * ctb], f16, name="opt", tag="opt")
                    OPv = OP[:].rearrange("p (n k) -> p k n", k=2)
                    nc.scalar.activation(OPv, Wv, AF.Exp, scale=ki0 / 3.0)
                    L = pb.tile([P, ctb], f16, name="L", tag="L")
                    eng = nc.gpsimd if pool_L else nc.vector
                    eng.tensor_tensor(L[:], dd[:][:, 0:ctb],
                                      dd[:][:, ctb:2 * ctb], Add)
                    O2 = iop.tile([P, ctb], f16, name="o2t", tag="o2t")
                    nc.vector.tensor_scalar_mul(O2[:], L[:], ki1 * 0.5)
                    nc.sync.dma_start(op_ap[:, bass.ts(j, 2 * ctb)], OP[:])
                    nc.sync.dma_start(o2_ap[:, bass.ts(j, ctb)], O2[:])

                for j in range(nB):
                    b_front(j)
                    if pipelined and j >= 1:
                        b_back(j - 1)
                    elif not pipelined:
                        b_back(j)
                if pipelined:
                    b_back(nB - 1)

    nc.compile()
    _cache[key] = nc
    return nc


def _prep_inputs(strain: np.ndarray):
    """strain [B, T, 3] f32 -> per-core fp16 planes."""
    B, T, C = strain.shape
    assert C == 3 and B % N_CORES == 0
    h = strain.astype(np.float16)
    hp = h.reshape(N_CORES, P, F, 3)
    a = np.ascontiguousarray(hp[..., 0])
    b = np.ascontiguousarray(hp[..., 1])
    c = np.ascontiguousarray(hp[..., 2])
    return a, b, c


def _assemble_out(op: np.ndarray, o2: np.ndarray, B: int, T: int):
    """op [8,P,2F] f16 pairs, o2 [8,P,F] f16 -> [B,T,3] f32."""
    out = np.empty((B, T, 3), dtype=np.float32)
    ov = out.reshape(N_CORES, P, F, 3)
    ov[..., 0:2] = op.reshape(N_CORES, P, F, 2)
    ov[..., 2] = o2
    return out


def _run(strain: np.ndarray, ki0: float, ki1: float, trace: bool = False,
         use_gpsimd: bool = True):
    B, T, C = strain.shape
    a, b, c = _prep_inputs(strain)
    nc = _build(float(ki0), float(ki1), 1, use_gpsimd)
    in_maps = [{"a": a[i], "b": b[i], "c": c[i]} for i in range(N_CORES)]
    res = run_bass_kernel_spmd(nc, in_maps, list(range(N_CORES)), trace=trace)
    op = np.stack([np.asarray(res.results[i]["op"]) for i in range(N_CORES)])
    o2 = np.stack([np.asarray(res.results[i]["o2"]) for i in range(N_CORES)])
    return _assemble_out(op, o2, B, T), res


def kernel(strain: np.ndarray, ki0, ki1) -> np.ndarray:
    out, _ = _run(np.asarray(strain), float(np.asarray(ki0)),
                  float(np.asarray(ki1)))
    return out


# revision 23
# speedup vs baseline: 1.6129x; 1.1357x over previous
"""Trainium2 Bass kernel for nn_KANStressPredictor (fp16 planes version).

Per element-triple (s0, s1, s2) of `strain` [B, T, 3]:
    t1, t2 = eigenvalues of C = (s0+s1+1) -/+ rad, rad = sqrt((s0-s1)^2+s2^2)
    out0, out1 = exp(ki0/3 * (l_i - 0.5*l_other)),  l_i = ln(t_i)
    out2       = ki1 * 0.5 * (l1 + l2)

Host-side: strain is recoded into three fp16 planes per core —
s = s0+s1, u = s0-s1, c = s2 (an orthogonal linear recode, same byte count)
— data-parallel over batch across 8 cores; device outputs fp16, host
upcasts to f32. Unit-stride 16-bit planes engage the DVE 2x performance
mode and halve HBM traffic vs f32.

Device structure per core ([128, 8192] planes, 2048-triple chunks):
  Phase A (sqrt table): DMA c/u/s chunk (s lands in its persistent tile);
      q = c*c (GPSIMD), u2 = u*u (DVE 2x), r2 = u2+q (DVE),
      rad = Sqrt(r2) (ACT). Each chunk's d-block [d1 || d2] = s -/+ rad
      is filled by DVE TTs immediately (the DVE has slack while input DMA
      paces the phase).
  Phase B (ln/exp table): l-block = Ln(d+1) in-place (one 4096-wide ACT
      op per block); h = -0.5*l_swapped (DVE TS 4x); w = h + l (DVE TT
      2x); o0/o1 = exp(ki0/3*w) in one ACT op writing interleaved pairs;
      L = l1+l2, o2 = 0.5*ki1*L (DVE); DMA out.

The ACT queue must see all Sqrt ops before all Ln/Exp ops (one
activation-table switch per pass, ~2.7us each): since the Tile scheduler
reorders by data-readiness, every Ln takes its bias (=1.0) from a fence
op Sqrt(0*x+1) that reads the last sqrt's output. All B-phase ln's are
enqueued before the first exp so the in-order ACT queue never stalls on
the DVE w-chain.
"""

import contextlib
import os
import sys

os.environ.setdefault("MYCRO_LOCAL_CACHE", "1")

for _p in ("/opt/trn_rl_repo",):
    if _p not in sys.path:
        sys.path.insert(0, _p)

import numpy as np

import concourse.bacc as bacc
import concourse.bass as bass
import concourse.tile as tile
from concourse import mybir
from concourse.bass_utils import run_bass_kernel_spmd

N_CORES = 8
P = 128
F = 8192            # triples per partition per core
CT = 2048           # phase-A chunk (triples)
CTB = 2048          # phase-B block (triples)

f32 = mybir.dt.float32
f16 = mybir.dt.float16
bf16 = mybir.dt.bfloat16

_cache: dict = {}


def _build(ki0: float, ki1: float, loop_reps: int = 1, use_gpsimd: bool = True,
           hoist: bool = True):
    key = (ki0, ki1, loop_reps, use_gpsimd, hoist)
    if key in _cache:
        return _cache[key]

    AF = mybir.ActivationFunctionType
    Add = mybir.AluOpType.add
    Sub = mybir.AluOpType.subtract
    Mult = mybir.AluOpType.mult

    n = F // CT          # A chunks
    nB = F // CTB        # B super chunks
    rA = CTB // CT       # A chunks per B super chunk

    nc = bacc.Bacc("TRN2", target_bir_lowering=False, debug=False)
    s_ap = nc.dram_tensor("s", [P, F], f16, kind="ExternalInput").ap()
    u_ap = nc.dram_tensor("u", [P, F], f16, kind="ExternalInput").ap()
    c_ap = nc.dram_tensor("c", [P, F], f16, kind="ExternalInput").ap()
    op_ap = nc.dram_tensor("op", [P, 2 * F], f16, kind="ExternalOutput").ap()
    o2_ap = nc.dram_tensor("o2", [P, F], f16, kind="ExternalOutput").ap()

    with tile.TileContext(nc) as tc:
        with (
            tc.tile_pool(name="persist", bufs=1) as pp,
            tc.tile_pool(name="pa", bufs=2) as pa,
            tc.tile_pool(name="pb", bufs=2) as pb,
            tc.tile_pool(name="pd", bufs=4) as pdd,
            tc.tile_pool(name="io", bufs=2) as iop,
        ):
            stk = [pp.tile([P, CT], f16, name=f"st{i}", tag=f"st{i}")
                   for i in range(n)]
            rdk = [pp.tile([P, CT], bf16, name=f"rd{i}", tag=f"rd{i}")
                   for i in range(n)]
            # phase-fence bias tile: each ln takes its bias (=1.0) from an
            # ACT op that reads the last Sqrt's output with scale=0, pinning
            # all sqrt-table work before all ln/exp work in the ACT queue.
            fence = pp.tile([P, 1], f32, name="fence", tag="fence")

            loop = tc.For_i(0, loop_reps) if loop_reps > 1 else \
                contextlib.nullcontext()
            with loop:
                # ---- Phase A (sqrt table) ----
                def a_chunk(ci):
                    sl = bass.ts(ci, CT)
                    U = iop.tile([P, CT], f16, name="u", tag="u")
                    C = iop.tile([P, CT], f16, name="c", tag="c")
                    nc.sync.dma_start(C[:], c_ap[:, sl])
                    nc.sync.dma_start(U[:], u_ap[:, sl])
                    nc.sync.dma_start(stk[ci][:], s_ap[:, sl])
                    q = pa.tile([P, CT], bf16, name="q", tag="q")
                    if use_gpsimd:
                        nc.gpsimd.tensor_tensor(q[:], C[:], C[:], Mult)
                    else:
                        nc.vector.tensor_tensor(q[:], C[:], C[:], Mult)
                    u2 = pa.tile([P, CT], bf16, name="u2", tag="u2")
                    nc.vector.tensor_tensor(u2[:], U[:], U[:], Mult)
                    r2 = pa.tile([P, CT], bf16, name="r2", tag="r2")
                    nc.vector.tensor_tensor(r2[:], u2[:], q[:], Add)
                    nc.scalar.activation(rdk[ci][:], r2[:], AF.Sqrt)

                # d super-block layout: [d1(sub0) | d1(sub1) | d2(sub0) | d2(sub1)]
                dds = {}

                def dd_fill(j):
                    dd = pdd.tile([P, 2 * CTB], bf16, name="dd", tag="dd")
                    dds[j] = dd
                    DD = dd[:]
                    for k in range(rA):
                        ca = j * rA + k
                        ST, RD = stk[ca][:], rdk[ca][:]
                        o1 = k * CT
                        o2_ = CTB + k * CT
                        nc.vector.tensor_tensor(DD[:, o1:o1 + CT], ST, RD,
                                                Sub)
                        nc.vector.tensor_tensor(DD[:, o2_:o2_ + CT], ST, RD,
                                                Add)

                for ci in range(n):
                    a_chunk(ci)
                    if hoist and (ci + 1) % rA == 0:
                        dd_fill(ci // rA)
                if not hoist:
                    for j in range(nB):
                        dd_fill(j)

                # phase fence: Sqrt(0*x + 1) = 1.0 reading the last sqrt's
                # output, so every ln (bias consumer) orders after all sqrts
                nc.scalar.activation(fence[:], rdk[n - 1][:][:, 0:1], AF.Sqrt,
                                     bias=1.0, scale=0.0)

                # ---- Phase B (ln/exp table) ----
                def b_ln(j):
                    dd = dds[j]
                    nc.scalar.activation(dd[:], dd[:], AF.Ln, bias=fence[:])

                def b_back(j):
                    dd = dds.pop(j)
                    ddv = dd[:].rearrange("p (k n) -> p k n", k=2)
                    lsw = ddv[:, ::-1]
                    H = pb.tile([P, 2 * CTB], bf16, name="h", tag="h")
                    Hv = H[:].rearrange("p (k n) -> p k n", k=2)
                    nc.vector.tensor_scalar_mul(Hv, lsw, -0.5)
                    W = pb.tile([P, 2 * CTB], bf16, name="w", tag="w")
                    Wv = W[:].rearrange("p (k n) -> p k n", k=2)
                    nc.vector.tensor_tensor(W[:], H[:], dd[:], Add)
                    OP = iop.tile([P, 2 * CTB], f16, name="opt", tag="opt")
                    OPv = OP[:].rearrange("p (n k) -> p k n", k=2)
                    nc.scalar.activation(OPv, Wv, AF.Exp, scale=ki0 / 3.0)
                    L = pb.tile([P, CTB], f16, name="L", tag="L")
                    nc.vector.tensor_tensor(L[:], dd[:][:, 0:CTB],
                                            dd[:][:, CTB:2 * CTB], Add)
                    O2 = iop.tile([P, CTB], f16, name="o2t", tag="o2t")
                    nc.vector.tensor_scalar_mul(O2[:], L[:], ki1 * 0.5)
                    nc.sync.dma_start(op_ap[:, bass.ts(j, 2 * CTB)], OP[:])
                    nc.sync.dma_start(o2_ap[:, bass.ts(j, CTB)], O2[:])

                for j in range(nB):
                    b_ln(j)
                for j in range(nB):
                    b_back(j)

    nc.compile()
    _cache[key] = nc
    return nc


def _prep_inputs(strain: np.ndarray):
    """strain [B, T, 3] f32 -> per-core fp16 planes (s=s0+s1, u=s0-s1, c)."""
    B, T, C = strain.shape
    assert C == 3 and B % N_CORES == 0
    sp = strain.reshape(N_CORES, P, F, 3)
    a = sp[..., 0]
    b = sp[..., 1]
    s = (a + b).astype(np.float16)
    u = (a - b).astype(np.float16)
    c = sp[..., 2].astype(np.float16)
    return {"s": s, "u": u, "c": c}


def _assemble_out(op: np.ndarray, o2: np.ndarray, B: int, T: int):
    """op [8,P,2F] f16 pairs, o2 [8,P,F] f16 -> [B,T,3] f32."""
    out = np.empty((B, T, 3), dtype=np.float32)
    ov = out.reshape(N_CORES, P, F, 3)
    ov[..., 0:2] = op.reshape(N_CORES, P, F, 2)
    ov[..., 2] = o2
    return out


def _run(strain: np.ndarray, ki0: float, ki1: float, trace: bool = False,
         use_gpsimd: bool = True):
    B, T, C = strain.shape
    planes = _prep_inputs(strain)
    nc = _build(float(ki0), float(ki1), 1, use_gpsimd)
    in_maps = [{k: v[i] for k, v in planes.items()} for i in range(N_CORES)]
    res = run_bass_kernel_spmd(nc, in_maps, list(range(N_CORES)), trace=trace)
    op = np.stack([np.asarray(res.results[i]["op"]) for i in range(N_CORES)])
    o2 = np.stack([np.asarray(res.results[i]["o2"]) for i in range(N_CORES)])
    return _assemble_out(op, o2, B, T), res


def kernel(strain: np.ndarray, ki0, ki1) -> np.ndarray:
    out, _ = _run(np.asarray(strain), float(np.asarray(ki0)),
                  float(np.asarray(ki1)))
    return out
